# revision 52
# baseline (speedup 1.0000x reference)
"""Decode-step attention-partition kernel for 8 Trainium2 NeuronCores.

Shape (hardcoded from the problem spec):
  x[16,1,4096], ln_w[4096], Wq/Wk/Wv/Wo[4096,4096],
  K_cache/V_cache[16,2048,32,128], cache_lens[16] int32.

Sharding: head-parallel. Core c owns heads [4c, 4c+4) for ALL 16 requests;
the host sums the 8 cores' o_proj partials and adds the residual (the
"all-reduce after o_proj" of the TP sharding).

Numerics: K/V caches and all weights are fp8 e3m4 (range +-15.5, 4 mantissa
bits; rel err ~1%). Weights are pre-scaled x64 on the host; the 1/64 comes
back for free: rstd is computed as rstd/64 (sqrt scale = 64^2/D = 1.0) so
xnt = xn/64 and q/k/v = xn @ W exactly; for o_proj the Z-reduction column
holds 64.0 instead of 1.0 so 1/Z absorbs Wo's x64. ln_w is folded into the
weights on the host.

DMA plan (each HWDGE ring executes one transfer at a time, so the three
issue paths are load-balanced):
  sync ring:   xT (64KB) -> wq (2MB) -> K cache (8 paired DMAs out of a
               host-packed contiguous layout, ~10.5MB, tail blocks folded
               in) -> out
  scalar ring: V cache (16 full-height DMAs, one per request, ~10.8MB)
  gpsimd/SWDGE: wk, wv, wo (6MB), 1/Z bounce, v-row splices

Matmul structure (measured: LDW[128x128]+MM(N=1) pairs sustain ~34ns):
  - x arrives host-pre-transposed (xT bf16); rstd comes from 32 PE
    gram-matmuls (diag of xT^T xT) + a broadcast outer product, then
    xnt = xT * bcast(rstd/64) on the DVE.
  - q/k projections W-stationary, transposed output [128e, 16b] directly.
  - v projection xnt-stationary in natural [16, 512] form (row splice).
  - Scores: K-block stationary fp8, q column moving; columns [128t, (b,h)].
  - V pass: V-block stationary fp8, p column moving; attn accumulates as
    columns [128d, (h,b)] feeding o_proj lhsT slices directly.

Request lengths are read on the host and baked into the instruction stream
(static trip counts, exact-size DMAs). Requests are sorted by length
descending so per-t-block "valid request" sets are prefixes.
"""

import sys
import types
import ctypes
import contextlib

import numpy as np
import ml_dtypes

BF16_NP = ml_dtypes.bfloat16
FP8_NP = ml_dtypes.float8_e3m4

# ---------------------------------------------------------------------------
# axon NTFF profile hook (the image's antenv lacks axon_hooks; the capability
# exists in libaxon_pjrt.so). Registered before concourse.bass_utils import.
# ---------------------------------------------------------------------------


def _install_ntff_hook():
    if "antenv.axon_hooks" in sys.modules:
        return
    try:
        lib = ctypes.CDLL("/opt/axon/libaxon_pjrt.so")
        lib.axon_start_nrt_profile.argtypes = [
            ctypes.POINTER(ctypes.c_int64),
            ctypes.c_size_t,
        ]
        lib.axon_start_nrt_profile.restype = ctypes.c_int64
        lib.axon_stop_nrt_profile.argtypes = [ctypes.c_char_p]
        lib.axon_stop_nrt_profile.restype = ctypes.c_int64
    except OSError:
        lib = None

    @contextlib.contextmanager
    def _hook(output_dir, device_ids):
        import jax

        jax.devices()
        if device_ids:
            ids = (ctypes.c_int64 * len(device_ids))(*device_ids)
            rc = lib.axon_start_nrt_profile(ids, len(device_ids))
        else:
            rc = lib.axon_start_nrt_profile(None, 0)
        if rc != 0:
            raise RuntimeError(f"axon_start_nrt_profile rc={rc}")
        try:
            yield
        finally:
            n = lib.axon_stop_nrt_profile(str(output_dir).encode())
            print(f"ntff profile: {n} file(s) -> {output_dir}", file=sys.stderr)

    mod = types.ModuleType("antenv.axon_hooks")
    mod.get_axon_ntff_profile_hook = (lambda: _hook) if lib is not None else (lambda: None)
    mod.set_axon_ntff_profile_hook = lambda h: None
    sys.modules["antenv.axon_hooks"] = mod


_install_ntff_hook()

import concourse.bass as bass
import concourse.mybir as mybir
import concourse.tile as tile
from concourse.vector_clock import ScopedClock
from concourse.masks import make_identity
from concourse.bass_utils import run_bass_kernel_spmd

# ---------------------------------------------------------------------------
# This walrus build rejects instructions with >1 semaphore wait command
# ("Too many sync wait commands" in setupSyncWait for CTRL structs). Tile's
# kernel-tail drain accumulates one wait per engine/DMA lane. Split the waits
# across preceding same-engine NOPs (1 wait each).
# ---------------------------------------------------------------------------
_MAXW = 1


def _patched_drain_and_barrier(self, tick_clock, wait_clock):
    nc = self.nc
    probe = nc.sync.nop(nofuse=True)
    wait_clock.add_sem_waits(probe.ins, ScopedClock({None: tick_clock.global_clock}))
    si = probe.ins.sync_info
    waits = list(si.on_wait) if si is not None else []
    if len(waits) > _MAXW:
        si.on_wait = waits[:_MAXW]
        for i in range(_MAXW, len(waits), _MAXW):
            nop = nc.sync.nop(nofuse=True)
            nop.ins.sync_info = mybir.SyncInfo(
                on_wait=waits[i : i + _MAXW], on_update=[]
            )
    nc.sync.drain()
    nc.all_engine_barrier()
    assert self.sems is not None
    popped = nc._tile_sem_poison_stack.pop()
    assert popped is self._sem_poison
    nc.clear_and_free_semaphores(list(self.sems.allocated().values()))
    nc.all_engine_barrier()


tile.TileContext._drain_and_barrier = _patched_drain_and_barrier

_wsplit_counter = [0]


def _split_excess_waits(nc):
    """Post-pass: this walrus build allows at most 1 sem-wait per instruction.
    Move excess waits onto preceding same-engine NoOps (same-engine program
    order preserves the wait semantics)."""
    for fn in nc.m.functions:
        for bb in fn.blocks:
            out = []
            changed = False
            for inst in bb.instructions:
                si = inst.sync_info
                if (
                    si is not None
                    and len(si.on_wait) > 1
                    and not isinstance(inst, mybir.InstAllEngineBarrier)
                ):
                    waits = list(si.on_wait)
                    for w in waits[:-1]:
                        _wsplit_counter[0] += 1
                        out.append(
                            mybir.InstNoOp(
                                name=f"I-wsplit-{_wsplit_counter[0]}",
                                engine=inst.engine,
                                sync_info=mybir.SyncInfo(
                                    on_wait=[w], on_update=[]
                                ),
                            )
                        )
                    si.on_wait = [waits[-1]]
                    changed = True
                out.append(inst)
            if changed:
                bb.instructions[:] = out

# ---------------------------------------------------------------------------

F32 = mybir.dt.float32
BF16 = mybir.dt.bfloat16
FP8 = mybir.dt.float8e3
P = 128
B = 16
T = 2048
D = 4096
H = 32
HD = 128
NHL = 4          # heads per core
NCORES = 8
EPS = 1e-6
NKC = D // P     # 32 contraction chunks for the projections
SCALE = 1.0 / float(np.sqrt(HD))
NJJ = T // P     # 16 t-blocks max
WS = 64.0        # host-side weight pre-scale (folded out via rstd and 1/Z)


def _build(Ls):
    """Build the per-core Bass kernel. Ls: 16 request lengths, sorted desc."""
    nblk = [l // P + 1 for l in Ls]          # t-blocks incl. the new token
    r = [l % P for l in Ls]                  # new-token row within tail block
    vt = [rr + 1 for rr in r]                # valid rows in tail block
    jmax = max(nblk)
    # tile j is touched by requests [0, nbj[j]) (lengths sorted descending)
    nbj = [sum(1 for b in range(B) if nblk[b] > j) for j in range(jmax)]
    # packed-K geometry (exact token counts L+1, not padded to blocks):
    # per pair i, requests (2i, 2i+1) concatenated
    nbt = [n * P for n in nblk]   # padded: keeps K block offsets 128-aligned
    seg_cols = [NHL * (nbt[2 * i] + nbt[2 * i + 1]) for i in range(B // 2)]
    seg_off = np.cumsum([0] + seg_cols).tolist()
    CT = seg_off[-1]
    voff = np.cumsum([0] + [nblk[b] * NHL * HD for b in range(B)]).tolist()

    def kblock(b, h, j):
        """column range start of block j of head h of request b in its seg."""
        off = NHL * nbt[b - 1] if b % 2 else 0
        return off + h * nbt[b] + j * P

    nc = bass.Bass()
    xt_d = nc.dram_tensor("xt", [P, NKC * B], BF16, kind="ExternalInput")
    wqa_d = nc.dram_tensor("wqa", [P, NKC * NHL * HD // 2], FP8, kind="ExternalInput")
    wqb_d = nc.dram_tensor("wqb", [P, NKC * NHL * HD // 2], FP8, kind="ExternalInput")
    wk_d = nc.dram_tensor("wk", [P, NKC * NHL * HD], FP8, kind="ExternalInput")
    wv_d = nc.dram_tensor("wv", [P, NKC * NHL * HD], FP8, kind="ExternalInput")
    wo_d = nc.dram_tensor("wo", [P, NHL * D], FP8, kind="ExternalInput")
    ktp_d = nc.dram_tensor("ktp", [P, CT], FP8, kind="ExternalInput")
    vcp_d = nc.dram_tensor("vcp", [P, sum(nblk) * NHL * HD], FP8,
                           kind="ExternalInput")
    out_d = nc.dram_tensor("out", [P, NKC * B], F32, kind="ExternalOutput")

    with tile.TileContext(nc) as tc:
        with (
            tc.tile_pool(name="const", bufs=1) as const_pool,
            tc.tile_pool(name="persist", bufs=1) as persist,
            tc.tile_pool(name="pcols", bufs=1) as p_pool,
            tc.tile_pool(name="wbig", bufs=1) as w_pool,
        ):
            identity = const_pool.tile([B, B], F32, tag="identity")
            make_identity(nc, identity[:])
            # Z-reduction column carries WS so 1/Z absorbs Wo's x64 pre-scale
            ones_col = const_pool.tile([P, 1], BF16, tag="ones")
            nc.gpsimd.memset(ones_col[:], WS)
            ones_row = const_pool.tile([1, P], F32, tag="onesr")
            nc.gpsimd.memset(ones_row[:], 1.0)
            zrow = const_pool.tile([1, 512], F32, tag="zrow")
            nc.gpsimd.memset(zrow[:], 0.0)

            # ---------------- persistent SBUF ----------------
            xt_sb = persist.tile([P, NKC * B], BF16, tag="xt")   # col kc*16+b
            xnt_sb = persist.tile([P, NKC * B], BF16, tag="xnt")
            qt_sb = persist.tile([P, NHL * B], BF16, tag="qt")   # col h*16+b
            kt_sb = persist.tile([P, NHL * B], BF16, tag="kt")   # col h*16+b
            v8_sb = persist.tile([B, NHL * HD], FP8, tag="v8")
            kseg = [
                persist.tile([P, seg_cols[i]], FP8, tag=f"kseg{i}",
                             name=f"kseg{i}")
                for i in range(B // 2)
            ]

            # ---------------- score / attn PSUM ----------------
            zatt_cm = tc.tile_pool(name="zatt", bufs=1, space="PSUM")
            zatt_pool = zatt_cm.__enter__()
            sc_cm = tc.tile_pool(name="sc", bufs=1, space="PSUM")
            sc_pool = sc_cm.__enter__()
            n_sc_banks = (jmax + 7) // 8
            sc_ps = [
                sc_pool.tile([P, 512], F32, tag=f"sc{i}", name=f"sc{i}")
                for i in range(n_sc_banks)
            ]
            z_ps = zatt_pool.tile([1, B * NHL], F32, tag="z")
            attn_ps = zatt_pool.tile([P, B * NHL], F32, tag="attn")

            def sc_slice(j, c0, c1, p0, p1):
                return sc_ps[j // 8][p0:p1, (j % 8) * 64 + c0 : (j % 8) * 64 + c1]

            for t_ in sc_ps:
                nc.vector.memset(t_[:], -1.0e30)
            nc.tensor.matmul(
                z_ps[:], zrow[0:1, 0:1], zrow[0:1, : B * NHL],
                start=True, stop=True, skip_group_check=True,
            )
            nc.tensor.matmul(
                attn_ps[:], zrow[0:1, :P], zrow[0:1, : B * NHL],
                start=True, stop=True, skip_group_check=True,
            )

            qk_cm = tc.tile_pool(name="qkps", bufs=1, space="PSUM")
            qkps = qk_cm.__enter__()
            vps_cm = tc.tile_pool(name="vps", bufs=1, space="PSUM")
            vpsp = vps_cm.__enter__()
            # weights (fp8, d-major layout: 16KB/partition runs); wq/wk
            # live in a pool that closes after the projections so the V
            # tiles can reuse their SBUF space
            wqk_cm = tc.tile_pool(name="wqk", bufs=1)
            wqk_pool = wqk_cm.__enter__()
            wqa_sb = wqk_pool.tile([P, NKC * NHL * HD // 2], FP8, tag="wqa")
            wqb_sb = wqk_pool.tile([P, NKC * NHL * HD // 2], FP8, tag="wqb")
            wk_sb = wqk_pool.tile([P, NKC * NHL * HD], FP8, tag="wk")
            wv_sb = wqk_pool.tile([P, NKC * NHL * HD], FP8, tag="wv")
            wo_sb = w_pool.tile([P, NHL * D], FP8, tag="wo")
            wqav = wqa_sb[:].rearrange("p (kc e) -> p kc e", e=NHL * HD // 2)
            wqbv = wqb_sb[:].rearrange("p (kc e) -> p kc e", e=NHL * HD // 2)
            wkv = wk_sb[:].rearrange("p (kc e) -> p kc e", e=NHL * HD)
            wvv = wv_sb[:].rearrange("p (kc e) -> p kc e", e=NHL * HD)
            wov = wo_sb[:].rearrange("p (hc e) -> p hc e", e=D)

            # ---- DMA issue plan: each ring is an ordered FIFO of
            # transfers; keeping only ~one transfer per ring in flight
            # preserves completion ordering (round-robin engines otherwise
            # finish everything late together).
            #   sync:   xt, wq(2 halves), K pairs 0-7, out
            #   scalar: wk, wv, V pairs 0,2,4,6
            #   gpsimd: V1, wo, v-row splices, V pairs 3,5,7
            nc.sync.dma_start(xt_sb[:], xt_d[:, :])
            nc.sync.dma_start(wqa_sb[:], wqa_d[:, :])
            nc.sync.dma_start(wqb_sb[:], wqb_d[:, :])
            for i in range(B // 2):
                nc.sync.dma_start(
                    kseg[i][:], ktp_d[:, seg_off[i] : seg_off[i + 1]]
                )
            nc.scalar.dma_start(wk_sb[:], wk_d[:, :])
            nc.scalar.dma_start(wv_sb[:], wv_d[:, :])

            # ---------------- rstd/WS via PE gram diag ----------------
            with (
                tc.tile_pool(name="rsps", bufs=1, space="PSUM") as rsps,
                tc.tile_pool(name="rssb", bufs=1) as rssb,
            ):
                rbank = rsps.tile([P, 48], F32, tag="rbank")
                gram = rbank[:B, 0:B]
                for kc in range(NKC):
                    nc.tensor.matmul(
                        gram,
                        xt_sb[:, kc * B : (kc + 1) * B],
                        xt_sb[:, kc * B : (kc + 1) * B],
                        start=(kc == 0), stop=(kc == NKC - 1),
                        skip_group_check=True,
                    )
                ssq = rssb.tile([B, 1], F32, tag="ssq")
                junk = rssb.tile([B, B], F32, tag="junk")
                nc.vector.scalar_tensor_tensor(
                    junk[:], gram, 1.0, identity[:],
                    mybir.AluOpType.bypass, mybir.AluOpType.mult,
                    accum_out=ssq[:],
                )
                ssq2 = rssb.tile([B, 1], F32, tag="ssq2")
                nc.vector.tensor_scalar_add(ssq2[:], ssq[:], EPS * D)
                # sqrt scale WS^2/D (= 1.0 here) -> this is rstd/WS
                std = rssb.tile([B, 1], F32, tag="std")
                nc.scalar.activation(
                    std[:], ssq2[:], mybir.ActivationFunctionType.Sqrt,
                    scale=WS * WS / D,
                )
                rstd = rssb.tile([B, 1], F32, tag="rstd")
                nc.vector.reciprocal(rstd[:], std[:])
                # broadcast rstd/WS to [128, 16]: row-ify then outer product
                rrow_ps = rbank[0:1, B : 2 * B]
                nc.tensor.matmul(rrow_ps, rstd[:], identity[:],
                                 start=True, stop=True, skip_group_check=True)
                rrow = rssb.tile([1, B], F32, tag="rrowsb")
                nc.scalar.copy(rrow[:], rrow_ps)
                bc_ps = rbank[:, 2 * B : 3 * B]
                nc.tensor.matmul(bc_ps, ones_row[:], rrow[:],
                                 start=True, stop=True, skip_group_check=True)
                bcast = persist.tile([P, B], F32, tag="bcast")
                nc.scalar.copy(bcast[:], bc_ps)

            # xnt = xT * bcast (per-column rstd/WS), chunks on the DVE
            for kc in range(NKC):
                nc.vector.scalar_tensor_tensor(
                    xnt_sb[:, kc * B : (kc + 1) * B],
                    xt_sb[:, kc * B : (kc + 1) * B],
                    1.0, bcast[:],
                    mybir.AluOpType.bypass, mybir.AluOpType.mult,
                )


            # ---------------- q/k projections (W-stationary) ----------------
            def proj_t(wslice, dst_sb, nm):
                prj = qkps.tile([P, NHL * B], F32, tag="qk", name=nm)
                for e in range(NHL):
                    for kc in range(NKC):
                        nc.tensor.matmul(
                            prj[:, e * B : (e + 1) * B],
                            wslice(kc, e),
                            xnt_sb[:, kc * B : (kc + 1) * B],
                            start=(kc == 0), stop=(kc == NKC - 1),
                            skip_group_check=True,
                        )
                    nc.scalar.copy(dst_sb[:, e * B : (e + 1) * B],
                                   prj[:, e * B : (e + 1) * B])

            def wq_slice(kc, e):
                v = wqav if e < 2 else wqbv
                return v[:, kc, (e % 2) * HD : (e % 2 + 1) * HD]

            def wk_slice(kc, e):
                return wkv[:, kc, e * HD : (e + 1) * HD]

            def emit_kproj():
                proj_t(wk_slice, kt_sb, "prk")

            def emit_vproj():
                v_ps = vpsp.tile([B, NHL * HD], F32, tag="vps")
                for kc in range(NKC):
                    nc.tensor.matmul(
                        v_ps[:],
                        xnt_sb[:, kc * B : (kc + 1) * B],
                        wvv[:, kc, :],
                        start=(kc == 0), stop=(kc == NKC - 1),
                    )
                nc.scalar.copy(v8_sb[:], v_ps[:])

            proj_t(wq_slice, qt_sb, "prq")


            # ------- per-pair pipeline: scores -> exp -> Z -> V pass -------
            # pair i = requests (2i, 2i+1), matching the kseg DMAs. All V
            # traffic rides the scalar HWDGE ring (ordered, per request);
            # k/v projections are emitted after the first pairs' scores so
            # the PE does not stall on their (later-arriving) weights.
            p_cols = {}
            vpool_cm = tc.tile_pool(name="vpool", bufs=5)
            vpool = vpool_cm.__enter__()
            vtvs = {}
            for i in range(B // 2):
                pb = [2 * i, 2 * i + 1]
                jm = max(nblk[b] for b in pb)
                for b in pb:
                    v_tile = vpool.tile([P, NJJ * NHL * HD], FP8, tag="v")
                    vap = v_tile[:, : nblk[b] * NHL * HD]
                    veng = nc.scalar if b % 2 == 0 else nc.gpsimd
                    veng.dma_start(
                        vap,
                        vcp_d[:, voff[b] : voff[b] + nblk[b] * NHL * HD],
                    )
                    vtvs[b] = vap.rearrange("p (jj d) -> p jj d", d=NHL * HD)
                # scores: full blocks
                for b in pb:
                    for j in range(nblk[b] - 1):
                        for h in range(NHL):
                            col = b * NHL + h
                            nc.tensor.matmul(
                                sc_slice(j, col, col + 1, 0, P),
                                kseg[i][:, kblock(b, h, j) : kblock(b, h, j) + P],
                                qt_sb[:, h * B + b : h * B + b + 1],
                                start=True, stop=True,
                            )
                if i == 0:
                    emit_kproj()
                if i == 1:
                    emit_vproj()
                if i == 2:
                    nc.gpsimd.dma_start(wo_sb[:], wo_d[:, :])
                # tails (need kt): pair 0's are emitted in pair 1's section
                for ti in ([0, 1] if i == 1 else [i] if i >= 2 else []):
                    for b in (2 * ti, 2 * ti + 1):
                        jt = nblk[b] - 1
                        for h in range(NHL):
                            cs = kblock(b, h, jt)
                            nc.vector.tensor_copy(
                                kseg[ti][:, cs + r[b] : cs + r[b] + 1],
                                kt_sb[:, h * B + b : h * B + b + 1],
                            )
                            col = b * NHL + h
                            nc.tensor.matmul(
                                sc_slice(jt, col, col + 1, 0, vt[b]),
                                kseg[ti][:, cs : cs + vt[b]],
                                qt_sb[:, h * B + b : h * B + b + 1],
                                start=True, stop=True,
                            )
                    # exps after tails so every p tile is final
                    jmt = max(nblk[b] for b in (2 * ti, 2 * ti + 1))
                    for j in range(jmt):
                        pc = p_pool.tile([P, 2 * NHL], BF16, tag=f"p{ti}_{j}",
                                         name=f"p{ti}_{j}")
                        nc.scalar.activation(
                            pc[:],
                            sc_slice(j, 8 * ti, 8 * ti + 8, 0, P),
                            mybir.ActivationFunctionType.Exp,
                            scale=SCALE,
                        )
                        p_cols[(ti, j)] = pc
                    # Z per request half (handles differing nblk)
                    for b in (2 * ti, 2 * ti + 1):
                        lc0 = (b % 2) * NHL
                        for j in range(nblk[b]):
                            nc.tensor.matmul(
                                z_ps[0:1, b * NHL : b * NHL + NHL],
                                ones_col[:],
                                p_cols[(ti, j)][:, lc0 : lc0 + NHL],
                                start=False, stop=(j == nblk[b] - 1),
                                skip_group_check=True,
                            )
                    # V pass (unnormalized p; 1/Z folded at the attn evac)
                    for b in (2 * ti, 2 * ti + 1):
                        lc0 = (b % 2) * NHL
                        # splice the new token's v row (SWDGE, tiny)
                        nc.gpsimd.dma_start(
                            vtvs[b][r[b] : r[b] + 1, nblk[b] - 1, :],
                            v8_sb[b : b + 1, :],
                        )
                        for j in range(nblk[b]):
                            tail = j == nblk[b] - 1
                            m = vt[b] if tail else P
                            for h in range(NHL):
                                nc.tensor.matmul(
                                    attn_ps[:, b * NHL + h : b * NHL + h + 1],
                                    vtvs[b][:m, j, h * HD : (h + 1) * HD],
                                    p_cols[(ti, j)][:m, lc0 + h : lc0 + h + 1],
                                    start=False, stop=tail,
                                    skip_group_check=True,
                                )
            vpool_cm.__exit__(None, None, None)
            vps_cm.__exit__(None, None, None)
            qk_cm.__exit__(None, None, None)
            wqk_cm.__exit__(None, None, None)

            invz64 = persist.tile([1, B * NHL], F32, tag="invz64")
            nc.vector.reciprocal(invz64[:], z_ps[:])
            bz_ps = zatt_pool.tile([P, B * NHL], F32, tag="bzps")
            nc.tensor.matmul(bz_ps[:], ones_row[:], invz64[:],
                             start=True, stop=True, skip_group_check=True)
            bz_sb = persist.tile([P, B * NHL], F32, tag="bzsb")
            nc.scalar.copy(bz_sb[:], bz_ps[:])
            attn_sb = persist.tile([P, B * NHL], BF16, tag="attnsb")
            nc.vector.scalar_tensor_tensor(
                attn_sb[:], attn_ps[:], 1.0, bz_sb[:],
                mybir.AluOpType.bypass, mybir.AluOpType.mult,
            )
            sc_cm.__exit__(None, None, None)

            # ---------------- o_proj partial (W-stationary, fp8) -----------
            # attn is already 1/Z-normalized; output is produced transposed
            # (oT[128e-chunk, 16b] per chunk) and re-assembled on the host,
            # which also applies the 8-core reduction and the residual.
            attn_v = attn_sb[:].rearrange("p (b h) -> p b h", h=NHL)
            oT_sb = persist.tile([P, NKC * B], F32, tag="oT")
            with tc.tile_pool(name="ops", bufs=4, space="PSUM") as o_ps_pool:
                for ec in range(NKC):
                    o_ps = o_ps_pool.tile([P, B], F32, tag="ops")
                    for hc in range(NHL):
                        nc.tensor.matmul(
                            o_ps[:],
                            wov[:, hc, ec * P : (ec + 1) * P],
                            attn_v[:, :, hc],
                            start=(hc == 0), stop=(hc == NHL - 1),
                        )
                    nc.scalar.copy(oT_sb[:, ec * B : (ec + 1) * B], o_ps[:])
            nc.sync.dma_start(out_d[:, :], oT_sb[:])
            zatt_cm.__exit__(None, None, None)

    _split_excess_waits(nc)
    return nc


def _prep_inputs(x, ln_w, Wq, Wk, Wv, Wo, K_cache, V_cache, cache_lens):
    x = np.asarray(x, np.float32).reshape(B, D)
    ln_w = np.asarray(ln_w, np.float32)
    cache_lens = np.asarray(cache_lens, np.int32)
    perm = np.argsort(-cache_lens, kind="stable")
    Ls = [int(cache_lens[p]) for p in perm]
    nblk = [l // P + 1 for l in Ls]
    x_s = np.ascontiguousarray(x[perm])
    K4 = np.asarray(K_cache, np.float32).reshape(B, T, H, HD)
    V4 = np.asarray(V_cache, np.float32).reshape(B, T, H, HD)

    # xT: [16, 4096] -> [16, 32, 128] -> [128, 32, 16] (col = kc*16 + b)
    xt = np.ascontiguousarray(
        x_s.reshape(B, NKC, P).transpose(2, 1, 0)
    ).reshape(P, NKC * B).astype(BF16_NP)

    def w_prep(W, h0, esplit=None):
        # fold ln_w, scale x64, slice cols, d-major chunk layout
        ws = (ln_w[:, None] * np.asarray(W, np.float32))[
            :, h0 * HD : (h0 + NHL) * HD
        ] * WS
        a = ws.reshape(NKC, P, NHL * HD)
        if esplit == 0:
            a = a[:, :, : NHL * HD // 2]
        elif esplit == 1:
            a = a[:, :, NHL * HD // 2 :]
        n = a.shape[2]
        return np.ascontiguousarray(a.transpose(1, 0, 2)).reshape(
            P, NKC * n
        ).astype(FP8_NP)

    in_maps = []
    for c in range(NCORES):
        h0 = c * NHL
        wo_s = np.asarray(Wo, np.float32)[h0 * HD : (h0 + NHL) * HD, :] * WS
        wo_prep = np.ascontiguousarray(
            wo_s.reshape(NHL, HD, D).transpose(1, 0, 2)
        ).reshape(P, NHL * D).astype(FP8_NP)
        # packed K: per request [4h, 128d, nblk*128 t] -> [128, 4h, nbt]
        karr = K4[perm][:, :, h0 : h0 + NHL, :].transpose(0, 2, 3, 1)
        parts = [
            np.ascontiguousarray(
                karr[b, :, :, : nblk[b] * P].transpose(1, 0, 2)
            ).reshape(P, NHL * nblk[b] * P)
            for b in range(B)
        ]
        ktp = np.concatenate(parts, axis=1).astype(FP8_NP)
        # packed V: per request [nblk*128 t, 512] -> [128 tp, nblk jj, 512],
        # requests concatenated along the free axis (pair DMAs are then
        # fully contiguous)
        v_s = V4[perm][:, :, h0 : h0 + NHL, :].reshape(B, T, NHL * HD)
        vparts = [
            np.ascontiguousarray(
                v_s[b, : nblk[b] * P].reshape(nblk[b], P, NHL * HD)
                .transpose(1, 0, 2)
            ).reshape(P, nblk[b] * NHL * HD)
            for b in range(B)
        ]
        vcp = np.concatenate(vparts, axis=1).astype(FP8_NP)
        in_maps.append(
            {
                "xt": xt,
                "wqa": w_prep(Wq, h0, 0),
                "wqb": w_prep(Wq, h0, 1),
                "wk": w_prep(Wk, h0),
                "wv": w_prep(Wv, h0),
                "wo": wo_prep,
                "ktp": ktp,
                "vcp": vcp,
            }
        )
    return in_maps, Ls, perm, x_s


def _run(x, ln_w, Wq, Wk, Wv, Wo, K_cache, V_cache, cache_lens, trace=False):
    in_maps, Ls, perm, x_s = _prep_inputs(
        x, ln_w, Wq, Wk, Wv, Wo, K_cache, V_cache, cache_lens
    )
    nc = _build(Ls)
    # the axon-proxied runtime occasionally hits a transient
    # NRT_EXEC_UNIT_UNRECOVERABLE; retry a couple of times
    last_exc = None
    for _attempt in range(3):
        try:
            res = run_bass_kernel_spmd(
                nc, in_maps, core_ids=list(range(NCORES)), trace=trace
            )
            break
        except Exception as e:  # noqa: BLE001
            last_exc = e
            import time as _time

            _time.sleep(2.0)
    else:
        raise last_exc
    oT = np.zeros((P, NKC * B), np.float32)
    for c in range(NCORES):
        oT += res.results[c]["out"]
    # oT[dd, ec*16+b] = o[b, ec*128+dd]
    partial = oT.reshape(P, NKC, B).transpose(2, 1, 0).reshape(B, D)
    out_sorted = x_s + partial
    out = np.empty((B, D), np.float32)
    out[perm] = out_sorted
    return out.reshape(B, 1, D), res


def kernel(x, ln_w, Wq, Wk, Wv, Wo, K_cache, V_cache, cache_lens):
    out, _ = _run(x, ln_w, Wq, Wk, Wv, Wo, K_cache, V_cache, cache_lens)
    return out
